# revision 2
# baseline (speedup 1.0000x reference)
"""Slot-attention v2: ACT-bound design for the TimelineSim cost model.

Per-core structure (data-parallel over batch, one element per core):

  Loads: wq0/wk0/x/c0/bo as f32 DMAs on SP (startup critical path, x as one
  3D DMA); everything else as gpsimd cast-DMAs (f32 DRAM -> bf16 SBUF).
  Transposes: wq0/wk0/x/c0 on PE (f32); c1-7, Wq123/Wk123/Wv/Wo via DMA
  xbar transpose (bf16 SBUF->SBUF, zero PE cost).
  sim^T[j,i] = kh^T qh (PSUM f32, bf16 operands); exp on ACT, fused accum.
  attn@v REORIENTED: av[i-part, dh+1] = sum_j expT[j, i-cols]^T [v*invS|invS]
  -> moving operand 65 wide: 65 PE cycles per accumulation matmul instead
  of 512. Renormalizer r[i] is column 64; invr = 1/r is per PARTITION, so
  normalization is one tensor_scalar per i-tile into bf16 av_sb.
  av_sb -> outT via ONE xbar DMA per head (heads 0-5) or PE f32 transposes
  (heads 6-7: tail latency).  y = outT^T woT in PSUM waves (pairs 0+1 at
  h6, 2 at h7, 3 in tail) + bo broadcast.
"""

import os
import sys
from collections import deque

sys.path.insert(0, "/opt/trn_rl_repo")

import numpy as np

import concourse.bass as bass
import concourse.mybir as mybir
import concourse.tile as tile
from concourse import bacc
from concourse.bass_utils import run_bass_kernel_spmd
from concourse.masks import make_identity

B = 8
N = 1024
M = 1024
D = 512
H = 8
DH = 64
INNER = H * DH
SCALE = DH ** -0.5
P = 128

F32 = mybir.dt.float32
CDT = mybir.dt.bfloat16

Exp = mybir.ActivationFunctionType.Exp
Mult = mybir.AluOpType.mult
Add = mybir.AluOpType.add

BUDGET = int(os.environ.get("BUDGET", "1275"))
import json as _json
HBUD = {int(k): v for k, v in _json.loads(
    os.environ.get("HBUD", '{}')).items()}


def _drain_all(g):
    if g is not None:
        for _ in g:
            pass


class Fillers:
    """PE-cycle-budgeted FIFO of generators; front gen drains first."""

    def __init__(self):
        self.q = deque()

    def push(self, name, gen):
        self.q.append([name, gen])

    def force(self, name):
        for pair in list(self.q):
            if pair[0] == name:
                _drain_all(pair[1])
                self.q.remove(pair)

    def drain(self, budget):
        while budget > 0 and self.q:
            try:
                budget -= self.q[0][1].send(None)
            except StopIteration:
                self.q.popleft()

    def force_all(self):
        while self.q:
            pair = self.q.popleft()
            _drain_all(pair[1])


def build(nc: bass.Bass):
    x_d = nc.declare_dram_parameter("x", [N, D], F32, isOutput=False)
    c_d = nc.declare_dram_parameter("context", [M, D], F32, isOutput=False)
    wq_d = nc.declare_dram_parameter("Wq", [INNER, D], F32, isOutput=False)
    wk_d = nc.declare_dram_parameter("Wk", [INNER, D], F32, isOutput=False)
    wv_d = nc.declare_dram_parameter("Wv", [INNER, D], F32, isOutput=False)
    wo_d = nc.declare_dram_parameter("Wo", [D, INNER], F32, isOutput=False)
    bo_d = nc.declare_dram_parameter("bo", [D], F32, isOutput=False)
    out_d = nc.declare_dram_parameter("out", [N, D], F32, isOutput=True)

    with tile.TileContext(nc) as tc:
        with tc.tile_pool(name="const", bufs=1) as const, \
             tc.tile_pool(name="stage", bufs=1) as stage, \
             tc.tile_pool(name="work", bufs=1) as work, \
             tc.tile_pool(name="ld", bufs=1) as ld:
            ps = tc.alloc_tile_pool(name="ps", bufs=1, space="PSUM")
            _emit(nc, tc, const, stage, work, ld, ps,
                  x_d, c_d, wq_d, wk_d, wv_d, wo_d, bo_d, out_d)
            ps.release()
    return nc


def _emit(nc, tc, const, stage, work, ld, ps,
          x_d, c_d, wq_d, wk_d, wv_d, wo_d, bo_d, out_d):
    # ---- constants -------------------------------------------------------
    ident = const.tile([P, P], F32, tag="ident")
    make_identity(nc, ident[:, :])
    ones128 = const.tile([1, P], CDT, tag="ones128")
    nc.gpsimd.memset(ones128[:, :], 1.0)
    bo_s = const.tile([1, D], F32, tag="bo_s")
    bo_sb = const.tile([1, D], CDT, tag="bo_sb")
    bo_b = const.tile([P, D], F32, tag="bo_b")

    # ---- persistent stage tiles -----------------------------------------
    wTq = stage.tile([P, 4 * INNER], CDT, tag="wTq")
    wTk = stage.tile([P, 4 * INNER], CDT, tag="wTk")
    wTv = stage.tile([P, 4 * INNER], CDT, tag="wTv")
    woT = stage.tile([P, 4 * D], CDT, tag="woT")
    xT = stage.tile([P, 4 * N], CDT, tag="xT")
    cT = stage.tile([P, 4 * M], CDT, tag="cT")
    qT = [stage.tile([P, N], CDT, tag=f"qT{t}", name=f"qT{t}") for t in range(4)]
    kT = [stage.tile([P, M], CDT, tag=f"kT{t}", name=f"kT{t}") for t in range(4)]
    v = [stage.tile([P, INNER], CDT, tag=f"v{t}", name=f"v{t}") for t in range(8)]
    outT = [stage.tile([P, N], CDT, tag=f"outT{t}", name=f"outT{t}") for t in range(4)]
    y_acc = [stage.tile([P, D], F32, tag=f"y_acc{t}", name=f"y_acc{t}") for t in range(8)]

    def r3(t, a):
        return t[:, :].rearrange("p (a b) -> p a b", a=a)

    wTq3, wTk3, wTv3, woT3 = r3(wTq, 4), r3(wTk, 4), r3(wTv, 4), r3(woT, 4)
    xT3, cT3 = r3(xT, 4), r3(cT, 4)

    # ---- loads -----------------------------------------------------------
    wq0s = ld.tile([P, D], F32, tag="wq0s")
    wk0s = ld.tile([P, D], F32, tag="wk0s")
    xs = ld.tile([P, 8 * D], F32, tag="xs")
    cs = ld.tile([P, 8 * D], CDT, tag="cs")
    nc.sync.dma_start(wq0s[:, :], wq_d[0:P, :])
    nc.sync.dma_start(wk0s[:, :], wk_d[0:P, :])
    nc.sync.dma_start(
        xs[:, 0:4 * D].rearrange("p (t d) -> p t d", t=4),
        x_d[0:4 * P, :].rearrange("(t p) d -> p t d", p=P))
    nc.sync.dma_start(
        xs[:, 4 * D:8 * D].rearrange("p (t d) -> p t d", t=4),
        x_d[4 * P:8 * P, :].rearrange("(t p) d -> p t d", p=P))
    nc.gpsimd.dma_start(
        cs[:, 0:2 * D].rearrange("p (t d) -> p t d", t=2),
        c_d[0:2 * P, :].rearrange("(t p) d -> p t d", p=P))
    nc.gpsimd.dma_start(
        cs[:, 2 * D:4 * D].rearrange("p (t d) -> p t d", t=2),
        c_d[2 * P:4 * P, :].rearrange("(t p) d -> p t d", p=P))
    nc.gpsimd.dma_start(
        cs[:, 4 * D:8 * D].rearrange("p (t d) -> p t d", t=4),
        c_d[4 * P:8 * P, :].rearrange("(t p) d -> p t d", p=P))
    nc.sync.dma_start(bo_s[:, :], bo_d[None, :])

    # gpsimd cast loads (f32 DRAM -> bf16 SBUF)
    wq123 = ld.tile([P, 3 * D], CDT, tag="wq123")
    wk123 = ld.tile([P, 3 * D], CDT, tag="wk123")
    wvs = ld.tile([P, 4 * D], CDT, tag="wvs")
    wos = ld.tile([P, 4 * D], CDT, tag="wos")
    for dst, src, nt in ((wq123, wq_d[P:4 * P, :], 3),
                         (wk123, wk_d[P:4 * P, :], 3),
                         (wvs, wv_d[:, :], 4),
                         (wos, wo_d[:, :], 4)):
        nc.gpsimd.dma_start(
            dst[:, :].rearrange("p (t d) -> p t d", t=nt),
            src.rearrange("(t p) d -> p t d", p=P))

    # ---- prep: PE f32 transposes for wq0/wk0/x/c0 -----------------------
    identb = const.tile([P, P], CDT, tag="identb")
    nc.vector.tensor_copy(identb[:, :], ident[:, :])

    TRTAGS = (("sim", N, 2), ("sim", N, 2), ("av", N, 1))

    def tr_f32(src, dst3, tpos, eng):
        tag, sz, bufs = TRTAGS[tr_f32.n % len(TRTAGS)]
        tr_f32.n += 1
        pt = ps.tile([P, sz], F32, tag=tag, bufs=bufs, name=f"pt{tr_f32.n}")
        f32src = src.dtype == F32
        ptv = pt[:, :] if f32src else pt[:, :].bitcast(CDT)
        idv = ident[:, :] if f32src else identb[:, :]
        for dt_ in range(4):
            nc.tensor.transpose(ptv[:, dt_ * P:(dt_ + 1) * P],
                                src[:, dt_ * P:(dt_ + 1) * P], idv)
        cp = nc.scalar.copy if eng == "act" else nc.vector.tensor_copy
        cp(dst3[:, :, tpos * P:(tpos + 1) * P],
           ptv[:, 0:D].rearrange("p (a b) -> p a b", a=4))

    tr_f32.n = 0
    tr_f32(wq0s, wTq3, 0, "act")
    tr_f32(wk0s, wTk3, 0, "act")
    for nt in range(4):
        tr_f32(xs[:, nt * D:(nt + 1) * D], xT3, nt,
               "act" if nt < 2 else "dve")

    # qT[0] projection, ic0 as soon as x0-3 are transposed
    pq = ps.tile([P, D], F32, tag="misc", bufs=2, name="pq0")
    for dt_ in range(4):
        nc.tensor.matmul(
            pq[:, :],
            wTq[:, dt_ * INNER:dt_ * INNER + P],
            xT[:, dt_ * N:dt_ * N + 512],
            start=(dt_ == 0), stop=(dt_ == 3))
    nc.vector.tensor_copy(qT[0][:, 0:512], pq[:, :])
    for nt in range(4, 8):
        tr_f32(xs[:, nt * D:(nt + 1) * D], xT3, nt,
               "act" if nt % 2 == 0 else "dve")
    pq2 = ps.tile([P, D], F32, tag="misc", bufs=2, name="pq1")
    for dt_ in range(4):
        nc.tensor.matmul(
            pq2[:, :],
            wTq[:, dt_ * INNER:dt_ * INNER + P],
            xT[:, dt_ * N + 512:dt_ * N + 1024],
            start=(dt_ == 0), stop=(dt_ == 3))
    nc.vector.tensor_copy(qT[0][:, 512:1024], pq2[:, :])

    def kT0_half(ic):
        pk = ps.tile([P, D], F32, tag="misc", bufs=2, name=f"pk{ic}")
        for dt_ in range(4):
            nc.tensor.matmul(
                pk[:, :],
                wTk[:, dt_ * INNER:dt_ * INNER + P],
                cT[:, dt_ * M + ic * 512:dt_ * M + (ic + 1) * 512],
                start=(dt_ == 0), stop=(dt_ == 3))
        nc.vector.tensor_copy(kT[0][:, ic * 512:(ic + 1) * 512], pk[:, :])

    for nt in range(4):
        tr_f32(cs[:, nt * D:(nt + 1) * D], cT3, nt, "dve")
    kT0_half(0)
    for nt in range(4, 8):
        tr_f32(cs[:, nt * D:(nt + 1) * D], cT3, nt, "dve")
    kT0_half(1)

    # ---- xbar transposes (SP queue, dependency order) -------------------
    for et in range(1, 4):
        nc.sync.dma_start_transpose(
            wTq3[:, :, et * P:(et + 1) * P], wq123[:, (et - 1) * D:et * D])
        nc.sync.dma_start_transpose(
            wTk3[:, :, et * P:(et + 1) * P], wk123[:, (et - 1) * D:et * D])
    for et in range(4):
        nc.sync.dma_start_transpose(
            wTv3[:, :, et * P:(et + 1) * P], wvs[:, et * D:(et + 1) * D])
    for dt_ in range(4):
        nc.sync.dma_start_transpose(
            woT3[:, :, dt_ * P:(dt_ + 1) * P], wos[:, dt_ * D:(dt_ + 1) * D])

    # ---- filler generators ----------------------------------------------
    def pair_proj(et):
        for nm in ("q", "k"):
            wt, srcT, dstT, NN = ((wTq, xT, qT, N) if nm == "q"
                                  else (wTk, cT, kT, M))
            for ic in range(2):
                pp = ps.tile([P, D], F32, tag="misc", bufs=2,
                             name=f"pp{et}_{nm}_{ic}")
                for dt_ in range(4):
                    nc.tensor.matmul(
                        pp[:, :],
                        wt[:, dt_ * INNER + et * P:dt_ * INNER + (et + 1) * P],
                        srcT[:, dt_ * NN + ic * 512:dt_ * NN + (ic + 1) * 512],
                        start=(dt_ == 0), stop=(dt_ == 3))
                    yield 512
                nc.vector.tensor_copy(
                    dstT[et][:, ic * 512:(ic + 1) * 512], pp[:, :])

    def v_grp(mt):
        pv = ps.tile([P, D], F32, tag="misc", bufs=2, name=f"pv{mt}")
        for dt_ in range(4):
            nc.tensor.matmul(
                pv[:, :],
                cT[:, dt_ * M + mt * P:dt_ * M + (mt + 1) * P],
                wTv[:, dt_ * INNER:(dt_ + 1) * INNER],
                start=(dt_ == 0), stop=(dt_ == 3))
        nc.vector.tensor_copy(v[mt][:, :], pv[:, :])

    def bo_bcast():
        nc.vector.tensor_copy(bo_sb[:, :], bo_s[:, :])
        pbo = ps.tile([P, D], F32, tag="misc", bufs=2, name="pbo")
        nc.tensor.matmul(pbo[:, :], ones128[:, :], bo_sb[:, :],
                         start=True, stop=True)
        yield 512
        nc.vector.tensor_copy(bo_b[:, :], pbo[:, :])

    def y_wave(pairs, acc):
        for nt in range(8):
            py = ps.tile([P, D], F32, tag="misc", bufs=2,
                         name=f"py{pairs[0]}_{nt}")
            for i, p_ in enumerate(pairs):
                nc.tensor.matmul(
                    py[:, :],
                    outT[p_][:, nt * P:(nt + 1) * P],
                    woT[:, p_ * D:(p_ + 1) * D],
                    start=(i == 0), stop=(i == len(pairs) - 1))
                yield 512
            acc(nt, py)

    # ---- attention -------------------------------------------------------
    expT_live = {}
    vhs_live = {}

    def make_vhs(h, jts, invS):
        tiles = vhs_live.setdefault(h, [None] * 8)
        for jt in jts:
            vt = work.tile([P, DH + 1], CDT, tag=f"vhs{jt}", bufs=3,
                           name=f"vhs{h}_{jt}")
            nc.gpsimd.tensor_scalar_mul(
                vt[:, 0:DH], v[jt][:, h * DH:(h + 1) * DH], invS[:, jt:jt + 1])
            nc.gpsimd.tensor_copy(vt[:, DH:DH + 1], invS[:, jt:jt + 1])
            tiles[jt] = vt

    AVOFF = [it * 65 if it < 4 else 512 + (it - 4) * 65 for it in range(8)]

    def attn_head(h, on_half=None):
        et2, ro = h // 2, (h % 2) * DH
        av = ps.tile([P, N], F32, tag="av", bufs=1, name=f"av{h}")
        rb = work.tile([P, 8], F32, tag="rbuf", bufs=2, name=f"rb{h}")
        ivr = work.tile([P, 8], F32, tag="invr", bufs=2, name=f"ivr{h}")
        asb = work.tile([P, 512], CDT, tag="avsb", bufs=2, name=f"asb{h}")
        eT, vh = expT_live[h], vhs_live[h]
        for it in range(8):
            off = AVOFF[it]
            for jt in range(8):
                nc.tensor.matmul(
                    av[:, off:off + DH + 1],
                    eT[jt][:, it * P:(it + 1) * P],
                    vh[jt][:, :],
                    start=(jt == 0), stop=(jt == 7))
                yield 65
            if it == 3 or it == 7:
                gb = 0 if it == 3 else 1
                g = gb * 4
                base = 0 if it == 3 else 512
                nc.vector.tensor_copy(
                    rb[:, g:g + 4].rearrange("p (a b) -> p a b", b=1),
                    av[:, base:base + 260]
                      .rearrange("p (a b) -> p a b", b=65)[:, :, DH:DH + 1])
                nc.vector.reciprocal(ivr[:, g:g + 4], rb[:, g:g + 4])
                neng = nc.vector
                for it2 in range(g, g + 4):
                    neng.tensor_scalar_mul(
                        asb[:, it2 * DH:(it2 + 1) * DH],
                        av[:, AVOFF[it2]:AVOFF[it2] + DH],
                        ivr[:, it2:it2 + 1])
                    yield 30
                ptr = ps.tile([P, D], F32, tag="misc", bufs=2,
                              name=f"ptr{h}_{gb}")
                ptrb = ptr[:, :].bitcast(CDT)
                for it2 in range(4):
                    nc.tensor.transpose(
                        ptrb[0:DH, it2 * P:(it2 + 1) * P],
                        asb[:, (g + it2) * DH:(g + it2 + 1) * DH],
                        identb[:, :])
                    yield 128
                nc.vector.tensor_copy(
                    outT[et2][ro:ro + DH, gb * 512:(gb + 1) * 512],
                    ptrb[0:DH, 0:512])
                if on_half is not None:
                    on_half(gb)
        del expT_live[h], vhs_live[h]

    # ---- head loop -------------------------------------------------------
    fl = Fillers()

    def y01_acc(nt, py):
        nc.vector.tensor_tensor(y_acc[nt][:, :], py[:, :], bo_b[:, :], Add)

    def tail_half(gb):
        for nt in range(gb * 4, gb * 4 + 4):
            py = ps.tile([P, D], F32, tag="misc", bufs=2, name=f"py23_{nt}")
            for i, p_ in enumerate((2, 3)):
                nc.tensor.matmul(py[:, :], outT[p_][:, nt * P:(nt + 1) * P],
                                 woT[:, p_ * D:(p_ + 1) * D],
                                 start=(i == 0), stop=(i == 1))
            ysb = work.tile([P, D], F32, tag="ysb", bufs=4, name=f"ysb{nt}")
            nc.vector.tensor_tensor(ysb[:, :], py[:, :], y_acc[nt][:, :], Add)
            nc.sync.dma_start(out_d[nt * P:(nt + 1) * P, :], ysb[:, :])

    invS_prev = None
    for h in range(H):
        et2, ro = h // 2, (h % 2) * DH
        sT = work.tile([P, 8], F32, tag="sT", bufs=2, name=f"sT{h}")
        invS = work.tile([P, 8], F32, tag="invS", bufs=2, name=f"invS{h}")
        for jt in range(8):
            psim = ps.tile([P, N], F32, tag="sim", bufs=2,
                           name=f"psim{h}_{jt}")
            for ic in range(2):
                nc.tensor.matmul(
                    psim[:, ic * 512:(ic + 1) * 512],
                    kT[et2][ro:ro + DH, jt * P:(jt + 1) * P],
                    qT[et2][ro:ro + DH, ic * 512:(ic + 1) * 512],
                    start=True, stop=True)
            eT = work.tile([P, N], CDT, tag=f"expT{jt}", bufs=3,
                           name=f"expT{h}_{jt}")
            nc.scalar.activation(eT[:, :], psim[:, :], Exp, scale=SCALE,
                                 accum_out=sT[:, jt:jt + 1])
            expT_live.setdefault(h, [None] * 8)[jt] = eT
            if h == 1:
                v_grp(jt)
                if jt == 3:
                    make_vhs(0, range(0, 4), invS_prev)
                elif jt == 7:
                    make_vhs(0, range(4, 8), invS_prev)
            if h == 7:
                nc.vector.reciprocal(invS[:, jt:jt + 1], sT[:, jt:jt + 1])
                make_vhs(h, (jt,), invS)
                if jt == 7:
                    fl.push("attn7", attn_head(7, on_half=tail_half))
            elif jt == 3:
                nc.vector.reciprocal(invS[:, 0:4], sT[:, 0:4])
                if h >= 1:
                    make_vhs(h, range(0, 4), invS)
            elif jt == 7:
                nc.vector.reciprocal(invS[:, 4:8], sT[:, 4:8])
                if h >= 1:
                    make_vhs(h, range(4, 8), invS)
            if h == 0 and jt == 2:
                fl.push("pair1", pair_proj(1))
            fl.drain(HBUD.get(h, BUDGET))
        # end-of-head scheduling
        invS_prev = invS
        if h == 1:
            fl.force("pair1")
            fl.push("attn0", attn_head(0))
            fl.push("attn1", attn_head(1))
        elif h == 2:
            fl.force("attn0")
            fl.push("attn2", attn_head(2))
            fl.push("pair2", pair_proj(2))
        elif h == 3:
            fl.force("attn1")
            fl.force("attn2")
            fl.force("pair2")
            fl.push("attn3", attn_head(3))
            fl.push("pair3", pair_proj(3))
            fl.push("bo", bo_bcast())
        elif h == 4:
            fl.force("attn3")
            fl.push("attn4", attn_head(4))
        elif h == 5:
            fl.force("pair3")
            fl.force("attn4")
            fl.push("attn5", attn_head(5))
            fl.push("y01", y_wave((0, 1), y01_acc))
        elif h == 6:
            fl.force("attn5")
            fl.push("attn6", attn_head(6))
        elif h == 7:
            fl.force("attn6")

    # ---- tail: attn7 halves emit y23+stores via tail_half ----------------
    fl.force_all()


_CACHE = {}


def get_nc():
    if "nc" not in _CACHE:
        nc = bacc.Bacc("TRN2", target_bir_lowering=False, num_devices=B)
        build(nc)
        nc.compile()
        _CACHE["nc"] = nc
    return _CACHE["nc"]


def kernel(x, context, Wq, Wk, Wv, Wo, bo):
    nc = get_nc()
    w = {
        "Wq": np.ascontiguousarray(Wq, dtype=np.float32),
        "Wk": np.ascontiguousarray(Wk, dtype=np.float32),
        "Wv": np.ascontiguousarray(Wv, dtype=np.float32),
        "Wo": np.ascontiguousarray(Wo, dtype=np.float32),
        "bo": np.ascontiguousarray(bo, dtype=np.float32),
    }
    in_maps = [
        {"x": np.ascontiguousarray(x[b], dtype=np.float32),
         "context": np.ascontiguousarray(context[b], dtype=np.float32),
         **w}
        for b in range(B)
    ]
    res = run_bass_kernel_spmd(nc, in_maps, core_ids=list(range(B)))
    _CACHE["last"] = res
    return np.stack([res.results[b]["out"] for b in range(B)], axis=0)
